# revision 22
# baseline (speedup 1.0000x reference)
"""Trainium2 Bass kernel for nn_CustomConv2d_32538672234916.

out[b,o,h,w] = K - sum_{ci,kh,kw} exp(x_patch)*exp(w) + bias[o],  K = Cin*kh*kw = 576
i.e. out = (K + bias) - conv2d(exp(x) [1-padded], exp(weight), stride 1)

Sharding: data-parallel over batch B=16 across 8 cores (2 batches/core),
weights/bias replicated.

Per-core GEMM formulation: for each 8-row output strip (512 pixels), the
3x3-tap conv is 6 accumulating matmuls into one PSUM tile [Cout=128, 512]:
  - 3 matmuls with K=128: taps (kh=0,kw) and (kh=1,kw) packed along the
    contraction dim. SBUF holds exp(x) twice: partitions 0-63 = padded
    exp(x), partitions 64-127 = same shifted down one image row, so one
    access pattern feeds both taps.
  - 3 matmuls with K=64: taps (kh=2,kw) read from the unshifted half.
Matmul operands are bf16 (fp32 PSUM accumulation); the weight-stationary
taps-outer/strips-inner order keeps LDWEIGHTS off the critical path.
Measured steady state ~16-18us/core vs ~17.5us HBM-roofline (6.3MB/core
mandatory traffic at 360GB/s) -- the 'ridge' regime target.
"""
import sys
sys.path.insert(0, '/opt/trn_rl_repo')
import numpy as np

B, CIN, H, W = 16, 64, 64, 64
COUT = 128
NCORES = 8
BL = B // NCORES          # batches per core
PAD_W = W + 2             # 66
PAD_TOT = PAD_W * (H + 2) # 66*66 = 4356
KSUM = float(CIN * 9)     # 576
ROWS_PER_TILE = 8
NTILES = H // ROWS_PER_TILE  # 8 strips per image

_CACHE = {}


def _build(reps=1, order="strip", mm_dtype="f32r", diag="full"):
    from concourse import bacc, mybir
    from concourse.tile import TileContext

    f32 = mybir.dt.float32
    mmdt = {"f32r": mybir.dt.float32r, "bf16": mybir.dt.bfloat16}[mm_dtype]
    Exp = mybir.ActivationFunctionType.Exp

    nc = bacc.Bacc("TRN2", target_bir_lowering=False, debug=False)
    x_d = nc.dram_tensor("x", [BL, CIN, H, W], f32, kind="ExternalInput")
    wpair_d = nc.dram_tensor("wpair", [128, 3 * COUT], mmdt, kind="ExternalInput")
    wsing_d = nc.dram_tensor("wsing", [64, 3 * COUT], mmdt, kind="ExternalInput")
    bvec_d = nc.dram_tensor("bvec", [COUT, 1], f32, kind="ExternalInput")
    out_d = nc.dram_tensor("out", [BL, COUT, H, W], f32, kind="ExternalOutput")
    x_ap = x_d.ap()
    out_ap = out_d.ap()

    npsum = 4 if order == "strip" else 1

    with TileContext(nc) as tc:
        with tc.tile_pool(name="consts", bufs=1) as consts, \
             tc.tile_pool(name="xp", bufs=2) as xp, \
             tc.tile_pool(name="ep", bufs=2) as ep, \
             tc.tile_pool(name="rp", bufs=4) as rp, \
             tc.tile_pool(name="pp", bufs=npsum, space="PSUM") as pp:
            wpair_t = consts.tile([128, 3 * COUT], mmdt)
            wsing_t = consts.tile([64, 3 * COUT], mmdt)
            bv_t = consts.tile([COUT, 1], f32)

            def load_consts():
                nc.sync.dma_start(wpair_t[:], wpair_d.ap())
                nc.sync.dma_start(wsing_t[:], wsing_d.ap())
                nc.sync.dma_start(bv_t[:], bvec_d.ap())

            def epilogue(b, t, pt):
                res = rp.tile([COUT, ROWS_PER_TILE * W], f32, tag="res",
                              name=f"res_{b}_{t}")
                nc.vector.tensor_scalar(res[:], pt[:], -1.0, bv_t[:],
                                        mybir.AluOpType.mult,
                                        mybir.AluOpType.add)
                t_out = 0 if diag == "smallout" else t
                nc.sync.dma_start(
                    out_ap[b][:, t_out * ROWS_PER_TILE:(t_out + 1) * ROWS_PER_TILE, :],
                    res[:])

            ets = {}
            for i, b in enumerate([b for _ in range(reps) for b in range(BL)]):
                # raw (unpadded) x: one contiguous 16KB/partition DMA
                xt = xp.tile([CIN, H * W], f32, tag="xt", name=f"xt_{i}")
                nc.sync.dma_start(xt[:], x_ap[b].rearrange("c h w -> c (h w)"))
                if i == 0:
                    # after the first x DMA so x-data flows immediately
                    load_consts()
                xt3 = xt.rearrange("p (h w) -> p h w", w=W)
                bufslot = i % 2
                if bufslot not in ets:
                    # exp(pad)=1.0 cells are written once per buffer and
                    # persist (later iterations only rewrite the interiors)
                    et = ep.tile([128, PAD_TOT], mmdt, tag="et",
                                 name=f"et_{bufslot}")
                    ets[bufslot] = et
                    e3 = et.rearrange("p (h w) -> p h w", w=PAD_W)
                    nc.vector.memset(e3[0:64, 0, :], 1.0)          # top pad row
                    nc.vector.memset(e3[0:64, H + 1, :], 1.0)      # bottom pad row
                    nc.vector.memset(e3[0:64, 1:H + 1, 0], 1.0)    # left pad col
                    nc.vector.memset(e3[0:64, 1:H + 1, W + 1], 1.0)  # right pad col
                et = ets[bufslot]
                et3 = et.rearrange("p (h w) -> p h w", w=PAD_W)
                # exp into the padded layout (partitions 0-63 = Cin), then
                # copy shifted down one padded row into partitions 64-127
                # (tap kh=1 reads the copy at the kh=0 offsets). Both are
                # split in half so downstream matmuls can start early.
                HH = H // 2
                nc.scalar.activation(et3[0:CIN, 1:HH + 1, 1:W + 1],
                                     xt3[:, 0:HH, :], Exp)
                if diag != "nodup":
                    nc.sync.dma_start(et[64:128, 0:HH * PAD_W],
                                      et[0:64, PAD_W:(HH + 1) * PAD_W])
                nc.scalar.activation(et3[0:CIN, HH + 1:H + 1, 1:W + 1],
                                     xt3[:, HH:H, :], Exp)
                if diag != "nodup":
                    nc.sync.dma_start(et[64:128, HH * PAD_W:PAD_TOT - PAD_W],
                                      et[0:64, (HH + 1) * PAD_W:PAD_TOT])
                elif (i, bufslot) in ((0, 0), (1, 1)):
                    nc.vector.memset(et[64:128, :], 1.0)

                def rhs_pair(t, dx):
                    h0 = t * ROWS_PER_TILE
                    return et3[0:128, h0:h0 + ROWS_PER_TILE, dx:dx + W]

                def rhs_sing(t, dx):
                    h0 = t * ROWS_PER_TILE
                    return et3[0:64, h0 + 2:h0 + 2 + ROWS_PER_TILE, dx:dx + W]

                if order == "strip":
                    for t in range(NTILES):
                        pt = pp.tile([COUT, ROWS_PER_TILE * W], f32, tag="pt",
                                     name=f"pt_{i}_{t}")
                        for dx in range(3):
                            nc.tensor.matmul(
                                pt[:], wpair_t[:, dx * COUT:(dx + 1) * COUT],
                                rhs_pair(t, dx), start=(dx == 0), stop=False)
                        for dx in range(3):
                            nc.tensor.matmul(
                                pt[:], wsing_t[:, dx * COUT:(dx + 1) * COUT],
                                rhs_sing(t, dx), start=False, stop=(dx == 2))
                        epilogue(b, t, pt)
                else:  # taps outer within groups of G strips
                    G = {"tap": NTILES, "tapb": NTILES, "tap4": 4, "tap2": 2}[order]
                    ntag, nbuf = (4, 2) if order == "tapb" else (8, 1)
                    nsing = 0 if diag == "halfmm" else 3
                    for g0 in range(0, NTILES, G):
                        strips = range(g0, g0 + G)
                        pts = {t: pp.tile([COUT, ROWS_PER_TILE * W], f32,
                                          tag=f"pt{t % ntag}", bufs=nbuf,
                                          name=f"pt_{i}_{t}")
                               for t in strips}
                        for dx in range(3):
                            for t in strips:
                                nc.tensor.matmul(
                                    pts[t][:],
                                    wpair_t[:, dx * COUT:(dx + 1) * COUT],
                                    rhs_pair(t, dx), start=(dx == 0),
                                    stop=(dx == 2 and nsing == 0))
                                if dx == 2 and nsing == 0:
                                    epilogue(b, t, pts[t])
                        for dx in range(nsing):
                            for t in strips:
                                nc.tensor.matmul(
                                    pts[t][:],
                                    wsing_t[:, dx * COUT:(dx + 1) * COUT],
                                    rhs_sing(t, dx), start=False, stop=(dx == 2))
                                if dx == 2:
                                    epilogue(b, t, pts[t])
    nc.compile()
    return nc


def _prep_weights(weight, bias, mm_dtype="f32r"):
    # wpair[ci, dx*128+o] = exp(w[o,ci,0,dx]); wpair[64+ci, ...] = exp(w[o,ci,1,dx])
    ew = np.exp(weight.astype(np.float32))           # [COUT, CIN, 3, 3]
    wpair = np.empty((128, 3 * COUT), np.float32)
    wsing = np.empty((64, 3 * COUT), np.float32)
    for dx in range(3):
        wpair[0:64, dx * COUT:(dx + 1) * COUT] = ew[:, :, 0, dx].T
        wpair[64:128, dx * COUT:(dx + 1) * COUT] = ew[:, :, 1, dx].T
        wsing[:, dx * COUT:(dx + 1) * COUT] = ew[:, :, 2, dx].T
    if mm_dtype == "bf16":
        import ml_dtypes
        wpair = wpair.astype(ml_dtypes.bfloat16)
        wsing = wsing.astype(ml_dtypes.bfloat16)
    bvec = (KSUM + bias.astype(np.float32)).reshape(COUT, 1)
    return wpair, wsing, bvec


ORDER = "tap"
MM_DTYPE = "bf16"


def kernel(x, weight, bias):
    from concourse import bass_utils

    x = np.ascontiguousarray(np.asarray(x, dtype=np.float32))
    weight = np.asarray(weight, dtype=np.float32)
    bias = np.asarray(bias, dtype=np.float32)

    if "nc" not in _CACHE:
        _CACHE["nc"] = _build(order=ORDER, mm_dtype=MM_DTYPE)
    nc = _CACHE["nc"]

    wpair, wsing, bvec = _prep_weights(weight, bias, MM_DTYPE)
    in_maps = [
        {"x": x[c * BL:(c + 1) * BL], "wpair": wpair, "wsing": wsing, "bvec": bvec}
        for c in range(NCORES)
    ]
    res = bass_utils.run_bass_kernel_spmd(nc, in_maps, core_ids=list(range(NCORES)))
    return np.concatenate([r["out"] for r in res.results], axis=0)


# revision 27
# speedup vs baseline: 1.0209x; 1.0209x over previous
"""Trainium2 Bass kernel for nn_CustomConv2d_32538672234916.

out[b,o,h,w] = K - sum_{ci,kh,kw} exp(x_patch)*exp(w) + bias[o],  K = Cin*kh*kw = 576
i.e. out = (K + bias) - conv2d(exp(x) [1-padded], exp(weight), stride 1)

Sharding: data-parallel over batch B=16 across 8 cores (2 batches/core),
weights/bias replicated.

Per-core GEMM formulation: for each 8-row output strip (512 pixels), the
3x3-tap conv is 6 accumulating matmuls into one PSUM tile [Cout=128, 512]:
  - 3 matmuls with K=128: taps (kh=0,kw) and (kh=1,kw) packed along the
    contraction dim. SBUF holds exp(x) twice: partitions 0-63 = padded
    exp(x), partitions 64-127 = same shifted down one image row, so one
    access pattern feeds both taps.
  - 3 matmuls with K=64: taps (kh=2,kw) read from the unshifted half.
Matmul operands are bf16 (fp32 PSUM accumulation); the weight-stationary
taps-outer/strips-inner order keeps LDWEIGHTS off the critical path.
Measured steady state ~16-18us/core vs ~17.5us HBM-roofline (6.3MB/core
mandatory traffic at 360GB/s) -- the 'ridge' regime target.
"""
import sys
sys.path.insert(0, '/opt/trn_rl_repo')
import numpy as np

B, CIN, H, W = 16, 64, 64, 64
COUT = 128
NCORES = 8
BL = B // NCORES          # batches per core
PAD_W = W + 2             # 66
PAD_TOT = PAD_W * (H + 2) # 66*66 = 4356
KSUM = float(CIN * 9)     # 576
ROWS_PER_TILE = 8
NTILES = H // ROWS_PER_TILE  # 8 strips per image

_CACHE = {}


def _build(reps=1, order="strip", mm_dtype="f32r", diag="full", xin="narrow"):
    from concourse import bacc, mybir
    from concourse.tile import TileContext

    f32 = mybir.dt.float32
    mmdt = {"f32r": mybir.dt.float32r, "bf16": mybir.dt.bfloat16}[mm_dtype]
    Exp = mybir.ActivationFunctionType.Exp

    nc = bacc.Bacc("TRN2", target_bir_lowering=False, debug=False)
    x_d = nc.dram_tensor("x", [BL, CIN, H, W], f32, kind="ExternalInput")
    wpair_d = nc.dram_tensor("wpair", [128, 3 * COUT], mmdt, kind="ExternalInput")
    wsing_d = nc.dram_tensor("wsing", [64, 3 * COUT], mmdt, kind="ExternalInput")
    bvec_d = nc.dram_tensor("bvec", [COUT, 1], f32, kind="ExternalInput")
    out_d = nc.dram_tensor("out", [BL, COUT, H, W], f32, kind="ExternalOutput")
    x_ap = x_d.ap()
    out_ap = out_d.ap()

    npsum = 4 if order == "strip" else 1
    nstage = 3 if diag == "deep" else 2
    nres = 8 if diag == "deep" else 4

    with TileContext(nc) as tc:
        with tc.tile_pool(name="consts", bufs=1) as consts, \
             tc.tile_pool(name="xp", bufs=nstage) as xp, \
             tc.tile_pool(name="ep", bufs=nstage) as ep, \
             tc.tile_pool(name="rp", bufs=nres) as rp, \
             tc.tile_pool(name="pp", bufs=npsum, space="PSUM") as pp:
            wpair_t = consts.tile([128, 3 * COUT], mmdt)
            wsing_t = consts.tile([64, 3 * COUT], mmdt)
            bv_t = consts.tile([COUT, 1], f32)

            def load_consts():
                nc.sync.dma_start(wpair_t[:], wpair_d.ap())
                nc.sync.dma_start(wsing_t[:], wsing_d.ap())
                nc.sync.dma_start(bv_t[:], bvec_d.ap())

            def epilogue(b, t, pt):
                res = rp.tile([COUT, ROWS_PER_TILE * W], f32, tag="res",
                              name=f"res_{b}_{t}")
                nc.vector.tensor_scalar(res[:], pt[:], -1.0, bv_t[:],
                                        mybir.AluOpType.mult,
                                        mybir.AluOpType.add)
                t_out = 0 if diag == "smallout" else t
                nc.sync.dma_start(
                    out_ap[b][:, t_out * ROWS_PER_TILE:(t_out + 1) * ROWS_PER_TILE, :],
                    res[:])

            ets = {}
            HH = H // 2
            for i, b in enumerate([b for _ in range(reps) for b in range(BL)]):
                if xin == "wide":
                    # x spread over all 128 partitions (full 16 DMA ports):
                    # partition ci      <- x[b, ci, 0:32, :]
                    # partition 64+ci   <- x[b, ci, 32:64, :]
                    xt = xp.tile([128, HH * W], f32, tag="xt", name=f"xt_{i}")
                    nc.sync.dma_start(
                        xt[:], x_ap[b].rearrange("c (s h) w -> s c (h w)", s=2))
                else:
                    xt = xp.tile([CIN, H * W], f32, tag="xt", name=f"xt_{i}")
                    nc.sync.dma_start(xt[:],
                                      x_ap[b].rearrange("c h w -> c (h w)"))
                if i == 0:
                    # after the first x DMA so x-data flows immediately
                    load_consts()
                xt3 = xt.rearrange("p (h w) -> p h w", w=W)
                bufslot = i % nstage
                if bufslot not in ets:
                    # exp(pad)=1.0 cells are written once per buffer and
                    # persist (later iterations only rewrite the interiors)
                    et = ep.tile([128, PAD_TOT], mmdt, tag="et",
                                 name=f"et_{bufslot}")
                    ets[bufslot] = et
                    e3 = et.rearrange("p (h w) -> p h w", w=PAD_W)
                    nc.vector.memset(e3[0:64, 0, :], 1.0)          # top pad row
                    nc.vector.memset(e3[0:64, H + 1, :], 1.0)      # bottom pad row
                    nc.vector.memset(e3[0:64, 1:H + 1, 0], 1.0)    # left pad col
                    nc.vector.memset(e3[0:64, 1:H + 1, W + 1], 1.0)  # right pad col
                    if xin == "wide":
                        # half1 rows 32..63 pads are never rewritten either
                        nc.vector.memset(e3[64:128, HH:H, 0], 1.0)
                        nc.vector.memset(e3[64:128, HH:H, W + 1], 1.0)
                et = ets[bufslot]
                et3 = et.rearrange("p (h w) -> p h w", w=PAD_W)
                # half0 (partitions 0-63) = padded exp(x); half1 (64-127) =
                # same shifted down one padded row (tap kh=1 reads it at the
                # kh=0 offsets).
                if xin == "wide":
                    # exp stays same-partition; the two shifted-copy DMAs move
                    # the opposite quadrants across the partition halves.
                    nc.scalar.activation(et3[0:CIN, 1:HH + 1, 1:W + 1],
                                         xt3[0:64], Exp)
                    # half1 rows 0..31  <- half0 rows 1..32 (pads included)
                    nc.sync.dma_start(et[64:128, 0:HH * PAD_W],
                                      et[0:64, PAD_W:(HH + 1) * PAD_W])
                    nc.scalar.activation(et3[64:128, HH:H, 1:W + 1],
                                         xt3[64:128], Exp)
                    # half0 rows 33..64 <- half1 rows 32..63 (pads included)
                    nc.sync.dma_start(
                        et[0:64, (HH + 1) * PAD_W:(H + 1) * PAD_W],
                        et[64:128, HH * PAD_W:H * PAD_W])
                else:
                    nc.scalar.activation(et3[0:CIN, 1:HH + 1, 1:W + 1],
                                         xt3[:, 0:HH, :], Exp)
                    if diag != "nodup":
                        nc.sync.dma_start(et[64:128, 0:HH * PAD_W],
                                          et[0:64, PAD_W:(HH + 1) * PAD_W])
                    nc.scalar.activation(et3[0:CIN, HH + 1:H + 1, 1:W + 1],
                                         xt3[:, HH:H, :], Exp)
                    if diag != "nodup":
                        nc.sync.dma_start(et[64:128, HH * PAD_W:PAD_TOT - PAD_W],
                                          et[0:64, (HH + 1) * PAD_W:PAD_TOT])
                    elif (i, bufslot) in ((0, 0), (1, 1)):
                        nc.vector.memset(et[64:128, :], 1.0)

                def rhs_pair(t, dx):
                    h0 = t * ROWS_PER_TILE
                    return et3[0:128, h0:h0 + ROWS_PER_TILE, dx:dx + W]

                def rhs_sing(t, dx):
                    h0 = t * ROWS_PER_TILE
                    return et3[0:64, h0 + 2:h0 + 2 + ROWS_PER_TILE, dx:dx + W]

                if order == "strip":
                    for t in range(NTILES):
                        pt = pp.tile([COUT, ROWS_PER_TILE * W], f32, tag="pt",
                                     name=f"pt_{i}_{t}")
                        for dx in range(3):
                            nc.tensor.matmul(
                                pt[:], wpair_t[:, dx * COUT:(dx + 1) * COUT],
                                rhs_pair(t, dx), start=(dx == 0), stop=False)
                        for dx in range(3):
                            nc.tensor.matmul(
                                pt[:], wsing_t[:, dx * COUT:(dx + 1) * COUT],
                                rhs_sing(t, dx), start=False, stop=(dx == 2))
                        epilogue(b, t, pt)
                else:  # taps outer within groups of G strips
                    G = {"tap": NTILES, "tapb": NTILES, "tap4": 4, "tap2": 2}[order]
                    ntag, nbuf = (4, 2) if order == "tapb" else (8, 1)
                    nsing = 0 if diag == "halfmm" else 3
                    for g0 in range(0, NTILES, G):
                        strips = range(g0, g0 + G)
                        pts = {t: pp.tile([COUT, ROWS_PER_TILE * W], f32,
                                          tag=f"pt{t % ntag}", bufs=nbuf,
                                          name=f"pt_{i}_{t}")
                               for t in strips}
                        for dx in range(3):
                            for t in strips:
                                nc.tensor.matmul(
                                    pts[t][:],
                                    wpair_t[:, dx * COUT:(dx + 1) * COUT],
                                    rhs_pair(t, dx), start=(dx == 0),
                                    stop=(dx == 2 and nsing == 0))
                                if dx == 2 and nsing == 0:
                                    epilogue(b, t, pts[t])
                        for dx in range(nsing):
                            for t in strips:
                                nc.tensor.matmul(
                                    pts[t][:],
                                    wsing_t[:, dx * COUT:(dx + 1) * COUT],
                                    rhs_sing(t, dx), start=False, stop=(dx == 2))
                                if dx == 2:
                                    epilogue(b, t, pts[t])
    nc.compile()
    return nc


def _prep_weights(weight, bias, mm_dtype="f32r"):
    # wpair[ci, dx*128+o] = exp(w[o,ci,0,dx]); wpair[64+ci, ...] = exp(w[o,ci,1,dx])
    ew = np.exp(weight.astype(np.float32))           # [COUT, CIN, 3, 3]
    wpair = np.empty((128, 3 * COUT), np.float32)
    wsing = np.empty((64, 3 * COUT), np.float32)
    for dx in range(3):
        wpair[0:64, dx * COUT:(dx + 1) * COUT] = ew[:, :, 0, dx].T
        wpair[64:128, dx * COUT:(dx + 1) * COUT] = ew[:, :, 1, dx].T
        wsing[:, dx * COUT:(dx + 1) * COUT] = ew[:, :, 2, dx].T
    if mm_dtype == "bf16":
        import ml_dtypes
        wpair = wpair.astype(ml_dtypes.bfloat16)
        wsing = wsing.astype(ml_dtypes.bfloat16)
    bvec = (KSUM + bias.astype(np.float32)).reshape(COUT, 1)
    return wpair, wsing, bvec


ORDER = "tap"
MM_DTYPE = "bf16"


def kernel(x, weight, bias):
    from concourse import bass_utils

    x = np.ascontiguousarray(np.asarray(x, dtype=np.float32))
    weight = np.asarray(weight, dtype=np.float32)
    bias = np.asarray(bias, dtype=np.float32)

    if "nc" not in _CACHE:
        _CACHE["nc"] = _build(order=ORDER, mm_dtype=MM_DTYPE)
    nc = _CACHE["nc"]

    wpair, wsing, bvec = _prep_weights(weight, bias, MM_DTYPE)
    in_maps = [
        {"x": x[c * BL:(c + 1) * BL], "wpair": wpair, "wsing": wsing, "bvec": bvec}
        for c in range(NCORES)
    ]
    res = bass_utils.run_bass_kernel_spmd(nc, in_maps, core_ids=list(range(NCORES)))
    return np.concatenate([r["out"] for r in res.results], axis=0)
